# revision 25
# baseline (speedup 1.0000x reference)
"""LinearJointTSAttention Trainium2 kernel (v3, bf16 pipeline).

Data-parallel over the video (b) axis: 8 videos -> 8 NeuronCores, one video
(f*n = 16*196 = 3136 tokens) per core.  All matmul operands are bf16 (psum
accumulates fp32).

Per-core pipeline (T = 3136 tokens, C = 1024, H = 16 heads, e = 64):
  Pass 1 (per 512-token chunk; x/wkv DMAs are chunk-contiguous):
    - k|v token-major: x chunk-slice stationary, w_kv moving -> psum
      [128 tok, 512 ch]; DVE relu(k)+scale -> k_sb bf16; ACT copies v (+ ones
      column) -> v_sb bf16; per 2-head pair, kv/k_sum accumulate directly in
      a persistent PSUM region across all 25 token tiles.  start=True clears
      the WHOLE psum bank, so only the even pair of each bank issues it.
    - q channel-major: w_q stationary, x moving -> DVE relu+scale -> q_ch
      bf16 resident in SBUF (no recompute in pass 2).
    The q-chains of the last two chunks are deferred until after the final
    kv matmul so the kvd/ksb finalize (DVE) hides under them.
  Finalize: kv and k_sum block-diag masked into bf16 stationaries kvd and
  ksb_bc ([128,128] per d-chunk) via DVE ops reading PSUM.
  Pass 2 (per chunk, projection software-pipelined one chunk behind):
    - denominator-broadcast: ONE matmul per d with masked ksb_bc stationary
      yields the denominator replicated across each head's 64 partitions;
      z = reciprocal_approx_fast (fp32, ~18 bits; EPS=1e-6 is negligible
      against denominators >= ~3e3 so it is dropped).
    - numerator po = kvd^T q; DVE att = po * z -> bf16.
    - projection token-major; bias added by DVE during the PSUM->SBUF copy;
      DMA fp32 rows out.
"""

import os
import numpy as np

B, N, C = 128, 196, 1024
H, E = 16, 64
F = 16              # num_frames
NV = 8              # videos == cores
T = F * N           # 3136 tokens per video
SCALE = E ** -0.5   # 0.125
EPS = 1e-6
CHUNK = 512
CHUNKS = [(i * CHUNK, min(CHUNK, T - i * CHUNK)) for i in range((T + CHUNK - 1) // CHUNK)]
NTILES = sum((cw + 127) // 128 for _, cw in CHUNKS)

_cached = None


def _build_nc():
    import concourse.bass as bass
    import concourse.mybir as mybir
    import concourse.tile as tile
    from concourse import bacc
    from contextlib import ExitStack

    f32 = mybir.dt.float32
    bf16 = mybir.dt.bfloat16
    AF = mybir.ActivationFunctionType

    nc = bacc.Bacc("TRN2", target_bir_lowering=False, debug=False)

    xC = nc.dram_tensor("xC", [128, 8 * T], bf16, kind="ExternalInput").ap()
    wkvC = nc.dram_tensor("wkvC", [128, 4, 8, 512], bf16, kind="ExternalInput").ap()
    wqR = nc.dram_tensor("wqR", [128, 8, C], bf16, kind="ExternalInput").ap()
    wpR = nc.dram_tensor("wpR", [128, 8, C], bf16, kind="ExternalInput").ap()
    biasR = nc.dram_tensor("biasR", [128, C], f32, kind="ExternalInput").ap()
    maskR = nc.dram_tensor("maskR", [128, 128], bf16, kind="ExternalInput").ap()
    y = nc.dram_tensor("y", [T, C], f32, kind="ExternalOutput").ap()
    DEBUG = bool(os.environ.get("KDEBUG"))
    if DEBUG:
        dbg_kvd = nc.dram_tensor("dbg_kvd", [128, 8 * 128], f32, kind="ExternalOutput").ap()
        dbg_ksb = nc.dram_tensor("dbg_ksb", [128, 8 * 128], f32, kind="ExternalOutput").ap()

    with tile.TileContext(nc) as tc:
        ctx = ExitStack()
        ctx.enter_context(nc.allow_low_precision(reason="bf16 pipeline; emulated end-to-end rel err 4.4e-3 vs 2e-2 tolerance"))
        with ctx:
            singles = ctx.enter_context(tc.tile_pool(name="singles", bufs=1))
            xpool = ctx.enter_context(tc.tile_pool(name="xpool", bufs=2))
            kpool = ctx.enter_context(tc.tile_pool(name="kpool", bufs=2))
            vpool = ctx.enter_context(tc.tile_pool(name="vpool", bufs=2))

            # weights: k|v first (pass-1 critical path), then q, then proj
            wkv_sb = singles.tile([128, 4, 8, 512], bf16)
            nc.sync.dma_start(out=wkv_sb[:, 0], in_=wkvC[:, 0])
            xt0 = xpool.tile([128, 8, CHUNK], bf16, tag="xt")
            # first tile's token-slice arrives first so compute starts sooner
            nc.sync.dma_start(out=xt0[:, :, 0:128],
                              in_=xC[:, 0:8 * 512].rearrange("p (a b) -> p a b", b=512)[:, :, 0:128])
            nc.sync.dma_start(out=xt0[:, :, 128:512],
                              in_=xC[:, 0:8 * 512].rearrange("p (a b) -> p a b", b=512)[:, :, 128:512])
            for dch in range(1, 4):
                nc.sync.dma_start(out=wkv_sb[:, dch], in_=wkvC[:, dch])
            wq_sb = singles.tile([128, 8, C], bf16)
            wp_sb = singles.tile([128, 8, C], bf16)
            nc.sync.dma_start(out=wq_sb, in_=wqR)
            # wp/bias/mask are not needed until pass 2 — their DMAs are issued
            # at chunk 1 so they don't steal startup bandwidth from wkv/x0/wq
            bias_sb = singles.tile([128, C], f32)
            mask_sb = singles.tile([128, 128], bf16)

            # PE pre-warm: dummy matmuls spanning the startup DMA window keep
            # the HAM activity monitor busy so the first real matmuls run at
            # 2.4 GHz instead of cold 1.2 GHz.
            warm = singles.tile([128, 512], bf16)
            nc.vector.memset(warm, 0.25)

            # resident q (bf16, channel-major) for all T tokens
            q_ch = singles.tile([128, 8, T], bf16)

            # ---- pass 1: q + k|v + kv/ksum PSUM accumulation -----------------
            p1 = ExitStack()
            ps_kv = p1.enter_context(tc.tile_pool(name="ps_kv", bufs=1, space="PSUM"))
            ps_work = p1.enter_context(tc.tile_pool(name="ps_work", bufs=4, space="PSUM"))
            # per 2-head pair p: cols 0:128 = kv block, col 128 = k_sum;
            # pairs padded to 256 f32 so each stays inside one PSUM bank.
            kvps = ps_kv.tile([128, 8, 256], f32)

            # warm-up: 40 x 128-row dummy matmuls (~4us cold) — enough sustained
            # activity for the HAM to unthrottle, short enough to finish well
            # before the startup DMAs land (PE executes in order).
            ps_w = ps_work.tile([128, 512], f32, tag="ps")
            for i in range(40):
                nc.tensor.matmul(ps_w[:, 0:128], warm[:, 0:128], warm[:, 0:128],
                                 start=True, stop=True)

            kv_idx = [0]
            kv_queue = []   # pending single kv matmul closures
            kv_pend = []    # tiles whose kv matmuls are not yet enqueued
            mmctr = [0]

            def enqueue_kv():
                k_sb, v_sb, tsz = kv_pend.pop(0)
                i = kv_idx[0]
                kv_idx[0] += 1
                for p in range(8):
                    # start=True clears the WHOLE psum bank; pairs share banks
                    # (2 per 2KB bank), so only the even pair may start the
                    # bank — the odd pair writes onto the cleared region with
                    # accumulate semantics (has_written=0 -> plain write).
                    def mk(k_sb=k_sb, v_sb=v_sb, tsz=tsz, i=i, p=p):
                        nc.tensor.matmul(
                            kvps[:, p, 0:129],
                            k_sb[:tsz, p * 128:(p + 1) * 128],
                            v_sb[:tsz, p, :],
                            start=(i == 0 and p % 2 == 0),
                            stop=(i == NTILES - 1),
                            skip_group_check=True,
                        )
                    kv_queue.append(mk)

            def drip(every=4):
                # interleave one pending kv matmul between big-stream matmuls
                # so its LDWEIGHTS hides under the previous 512-row stream
                mmctr[0] += 1
                if kv_queue and mmctr[0] % every == 0:
                    kv_queue.pop(0)()

            def drain_kv():
                while kv_pend:
                    enqueue_kv()
                while kv_queue:
                    kv_queue.pop(0)()

            def q_chain(xt, t0, cw, d):
                ps = ps_work.tile([128, 512], f32, tag="ps")
                for c in range(8):
                    nc.tensor.matmul(
                        ps[:, :cw],
                        wq_sb[:, c, d * 128:(d + 1) * 128],
                        xt[:, c, :cw],
                        start=(c == 0), stop=(c == 7),
                    )
                    drip(every=2)
                nc.vector.tensor_scalar(
                    out=q_ch[:, d, t0:t0 + cw], in0=ps[:, :cw],
                    scalar1=SCALE, scalar2=SCALE,
                    op0=mybir.AluOpType.add, op1=mybir.AluOpType.max,
                )

            deferred = []
            for ci, (t0, cw) in enumerate(CHUNKS):
                if ci == 0:
                    xt = xt0
                else:
                    xt = xpool.tile([128, 8, CHUNK], bf16, tag="xt")
                    nc.sync.dma_start(
                        out=xt[:, :, :cw],
                        in_=xC[:, 8 * t0:8 * (t0 + cw)].rearrange("p (a b) -> p a b", b=cw),
                    )
                if ci == 1:  # late, non-critical loads
                    nc.sync.dma_start(out=wp_sb, in_=wpR)
                    nc.sync.dma_start(out=bias_sb, in_=biasR)
                    nc.sync.dma_start(out=mask_sb, in_=maskR)
                for tt in range(0, cw, 128):
                    tsz = min(128, cw - tt)
                    k_sb = kpool.tile([128, C], bf16, tag="k_sb")
                    v_sb = vpool.tile([128, 8, 129], bf16, tag="v_sb")
                    nc.vector.memset(v_sb[:tsz, :, 128:129], 1.0)
                    for dch in range(4):
                        ps = ps_work.tile([128, 512], f32, tag="ps")
                        for c in range(8):
                            nc.tensor.matmul(
                                ps[:tsz, :],
                                xt[:, c, tt:tt + tsz],
                                wkv_sb[:, dch, c, :],
                                start=(c == 0), stop=(c == 7),
                            )
                            if dch > 0:
                                drip()
                        if dch < 2:  # k: relu + scale offset
                            nc.vector.tensor_scalar(
                                out=k_sb[:tsz, dch * 512:(dch + 1) * 512],
                                in0=ps[:tsz, :],
                                scalar1=SCALE, scalar2=SCALE,
                                op0=mybir.AluOpType.add, op1=mybir.AluOpType.max,
                            )
                        else:  # v: copy on ACT engine (keeps DVE free)
                            p0 = (dch - 2) * 4
                            nc.scalar.activation(
                                out=v_sb[:tsz, p0:p0 + 4, 0:128],
                                in_=ps[:tsz, :].rearrange("p (a b) -> p a b", b=128),
                                func=AF.Copy,
                            )
                    kv_pend.append((k_sb, v_sb, tsz))
                    if len(kv_pend) >= 2:
                        enqueue_kv()
                if ci < len(CHUNKS) - 2:
                    while kv_pend:
                        enqueue_kv()
                    for d in range(8):
                        q_chain(xt, t0, cw, d)
                    while kv_queue:
                        kv_queue.pop(0)()
                else:
                    # defer q-chains so the kvd/ksb finalize hides under them
                    drain_kv()
                    deferred.append((xt, t0, cw))

            # ---- finalize (overlapped with deferred q-chains) ----------------
            ksum_sb = singles.tile([128, 8, 1], f32)
            kvd = singles.tile([128, 8, 128], bf16)
            ksb_bc = singles.tile([128, 8, 128], bf16)

            def finalize():
                nc.vector.tensor_copy(out=ksum_sb, in_=kvps[:, :, 128:129])
                for d in range(8):
                    nc.vector.tensor_mul(kvd[:, d, :], kvps[:, d, 0:128], mask_sb)
                    nc.vector.tensor_scalar(
                        out=ksb_bc[:, d, :], in0=mask_sb,
                        scalar1=ksum_sb[:, d, :], scalar2=None,
                        op0=mybir.AluOpType.mult,
                    )

            done_fin = False
            for (xt, t0, cw) in deferred:
                for d in range(8):
                    q_chain(xt, t0, cw, d)
                    if not done_fin:
                        finalize()
                        done_fin = True

            if DEBUG:
                dbg1 = singles.tile([128, 8, 128], f32)
                dbg2 = singles.tile([128, 8, 128], f32)
                nc.vector.tensor_copy(out=dbg1, in_=kvd)
                nc.vector.tensor_copy(out=dbg2, in_=ksb_bc)
                nc.sync.dma_start(out=dbg_kvd, in_=dbg1.rearrange("p a b -> p (a b)"))
                nc.sync.dma_start(out=dbg_ksb, in_=dbg2.rearrange("p a b -> p (a b)"))
            p1.close()

            # ---- pass 2: attention + projection ------------------------------
            zpool = ctx.enter_context(tc.tile_pool(name="zpool", bufs=3))
            attpool = ctx.enter_context(tc.tile_pool(name="attpool", bufs=2))
            ypool = ctx.enter_context(tc.tile_pool(name="ypool", bufs=3))
            ps_dn = ctx.enter_context(tc.tile_pool(name="ps_dn", bufs=3, space="PSUM"))
            ps_po = ctx.enter_context(tc.tile_pool(name="ps_po", bufs=2, space="PSUM"))
            ps_prj = ctx.enter_context(tc.tile_pool(name="ps_prj", bufs=3, space="PSUM"))

            def do_proj(att, t0, cw):
                for tt in range(0, cw, 128):
                    tsz = min(128, cw - tt)
                    yt = ypool.tile([128, C], f32, tag="yt")
                    for cc in range(2):
                        ps = ps_prj.tile([128, 512], f32, tag="prj")
                        for c in range(8):
                            nc.tensor.matmul(
                                ps[:tsz, :],
                                att[:, c, tt:tt + tsz],
                                wp_sb[:, c, cc * 512:(cc + 1) * 512],
                                start=(c == 0), stop=(c == 7),
                            )
                        nc.vector.tensor_add(
                            yt[:tsz, cc * 512:(cc + 1) * 512],
                            ps[:tsz, :],
                            bias_sb[:tsz, cc * 512:(cc + 1) * 512],
                        )
                    nc.sync.dma_start(out=y[t0 + tt:t0 + tt + tsz, :], in_=yt[:tsz, :])

            prev = None
            for ci, (t0, cw) in enumerate(CHUNKS):
                att = attpool.tile([128, 8, CHUNK], bf16, tag="att")
                for d in range(8):
                    dnb = ps_dn.tile([128, 512], f32, tag="dnb")
                    nc.tensor.matmul(
                        dnb[:, :cw], ksb_bc[:, d, :], q_ch[:, d, t0:t0 + cw],
                        start=True, stop=True,
                    )
                    z = zpool.tile([128, CHUNK], f32, tag="z")
                    nc.vector.reciprocal_approx_fast(out=z[:, :cw], in_=dnb[:, :cw])
                    po = ps_po.tile([128, 512], f32, tag="po")
                    nc.tensor.matmul(
                        po[:, :cw], kvd[:, d, :], q_ch[:, d, t0:t0 + cw],
                        start=True, stop=True,
                    )
                    nc.vector.tensor_mul(att[:, d, :cw], po[:, :cw], z[:, :cw])
                if prev is not None:
                    do_proj(*prev)
                prev = (att, t0, cw)
            do_proj(*prev)

    nc.compile()
    return nc


def _get_nc():
    global _cached
    if _cached is None:
        _cached = _build_nc()
    return _cached


def kernel(**inputs):
    import ml_dtypes

    x = np.asarray(inputs["x"], dtype=np.float32)
    w_qkv = np.asarray(inputs["w_qkv"], dtype=np.float32)
    w_proj = np.asarray(inputs["w_proj"], dtype=np.float32)
    b_proj = np.asarray(inputs["b_proj"], dtype=np.float32)

    from concourse.bass_utils import run_bass_kernel_spmd

    nc = _get_nc()

    bf = ml_dtypes.bfloat16
    wqkvT = np.ascontiguousarray(w_qkv.T).astype(bf)     # [C, 3C]
    # [C_in, out] -> [128, 8, out]: partition p holds channel c*128+p
    wqR = np.ascontiguousarray(wqkvT[:, 0:C].reshape(8, 128, C).transpose(1, 0, 2))
    wkvC = np.ascontiguousarray(
        wqkvT[:, C:3 * C].reshape(8, 128, 4, 512).transpose(1, 2, 0, 3))
    wpR = np.ascontiguousarray(w_proj.T.astype(bf).reshape(8, 128, C).transpose(1, 0, 2))
    biasR = np.ascontiguousarray(np.broadcast_to(b_proj, (128, C)))
    mask = np.zeros((128, 128), dtype=bf)
    mask[0:64, 0:64] = 1
    mask[64:128, 64:128] = 1

    in_maps = []
    for v in range(NV):
        xv = x[v * F:(v + 1) * F].reshape(T, C).T.astype(bf)   # [C, T]
        x8 = xv.reshape(8, 128, T).transpose(1, 0, 2)          # [128, 8, T]
        xCv = np.empty((128, 8 * T), dtype=bf)
        off = 0
        for t0, cw in CHUNKS:
            xCv[:, off:off + 8 * cw] = x8[:, :, t0:t0 + cw].reshape(128, 8 * cw)
            off += 8 * cw
        in_maps.append({
            "xC": xCv,
            "wkvC": wkvC,
            "wqR": wqR,
            "wpR": wpR,
            "biasR": biasR,
            "maskR": mask,
        })

    trace = bool(os.environ.get("KTRACE"))
    kw = {}
    if trace:
        import tempfile
        kw = dict(trace=True, tmpdir=tempfile.mkdtemp(prefix="ktrace_"))
    res = run_bass_kernel_spmd(nc, in_maps, core_ids=list(range(NV)), **kw)
    if trace:
        global last_result
        last_result = res
        last_result.tmpdir = kw["tmpdir"]
    out = np.empty((B, N, C), dtype=np.float32)
    for v in range(NV):
        out[v * F:(v + 1) * F] = res.results[v]["y"].reshape(F, N, C)
    return out
